# revision 26
# baseline (speedup 1.0000x reference)
"""VQ codebook assignment (ApplyKmeans) on 8 Trainium2 NeuronCores.

tokens[n] = argmin_k ||x_n - c_k||^2 = argmax_k (x_n.c_k - Cnorm_k/2)
(||x_n||^2 is constant per row). Data-parallel: x sharded along N
across 8 cores, C/Cnorm replicated.

Per core (16384 rows, 128 row-tiles of 128 rows):
  - host pre-tiles x^T so each [128d, 128n] stationary tile is
    contiguous (fp16: halves HBM traffic; PSUM accumulates fp32;
    52/131072 argmin flips vs the fp32 reference, rel err 0.0144)
  - per tile: 8 accumulating matmuls (x^T chunk stationary, C chunk
    moving) on top of a bias pre-load, so PSUM [128, 300] holds
    val = x.C - Cnorm/2
  - bias pre-load: tiles 0-7 matmul the bias in (ones x [-Cnorm/2
    hi/lo] rows, start=True) which sets every PSUM has_written bit;
    afterwards the otherwise-idle ScalarE rewrites the bank to the
    bias values and the start=False matmuls accumulate onto it -
    saving the PE a 9th matmul per tile
  - VectorE max8 + max_index -> first-occurrence argmax per row
  - tokens assembled in SBUF, streamed out in 16-tile blocks through
    t=112, then 8/6/2 so the final CAST+DMA on the kernel tail is tiny

Row interleaving: row-tile t holds rows {p*128 + t}, so the token
buffer [p, t] DMAs out contiguously in original row order.

Walrus only lowers one sync wait per instruction; _hoist_excess_waits
moves Tile's extra waits onto same-engine no-ops at the same program
point. x loads share the sync HWDGE ring (same-ring transfers complete
in order, so prefetch can't starve urgent loads); constants and token
stores ride the scalar ring.

Optimization notes from profiling (kept for future work): steady-state
PE is ~130ns/matmul vs a 127.2 floor; DVE (max8 478 + find_index8 469
+ match 91 per tile) is co-critical with the PE. A pair-FIND variant
(two tiles per 2-bank PSUM tile, one 600-wide FIND) reaches 127ns/MM
steady but loses ~1us net: startup is DMA-bandwidth-bound (~330GB/s
shared across rings, ~2.2us first-transfer latency after the ~7.5us
framework preamble), and this layout''s chunk-major group-0 +
bias-matmul start is better rate-matched to the arrival stream.
Engine clock varies run to run (2.4 vs 2.0 GHz throttle episodes:
check MATMUL median duration ~290ns vs ~348ns in the profile before
comparing timings).
"""
import os
import sys

import numpy as np

if "/opt/trn_rl_repo" not in sys.path:
    sys.path.insert(0, "/opt/trn_rl_repo")

import concourse.bass as bass
import concourse.mybir as mybir
import concourse.tile_sem_assignment as _tsa
from concourse.bass_utils import run_bass_kernel_spmd
from concourse.tile import TileContext

_tsa.NUM_HWDGE_SEMS = int(os.environ.get("KM_HW_LANES", "8"))

_orig_assign_tick = _tsa.TileClockTick._assign_tick


def _assign_tick_lanepools(self, inst):
    try:
        if isinstance(inst, _tsa.DMAInst) and inst.engine != mybir.EngineType.Pool:
            if not hasattr(self, "_lane_ctr"):
                self._lane_ctr = {}
            eng = inst.engine
            n = _tsa.NUM_HWDGE_SEMS
            half = max(1, n // 2)
            pool = (
                list(range(0, half))
                if eng == mybir.EngineType.Activation
                else list(range(half, n))
            )
            c = self._lane_ctr.get(eng, 0)
            self.next_hw_dma_idx = pool[c % len(pool)]
            self._lane_ctr[eng] = c + 1
    except Exception:
        pass
    return _orig_assign_tick(self, inst)


_tsa.TileClockTick._assign_tick = _assign_tick_lanepools

P = 128
D = 1024
K = 300
NCORES = 8
ROWS = 16384
TILES = ROWS // P
GROUPS = 32
TPG = TILES // GROUPS
DCH = D // P

F16 = mybir.dt.float16
F32 = mybir.dt.float32
I32 = mybir.dt.int32
U32 = mybir.dt.uint32

LAST_RESULT = None


def _ensure_ntff_hook():
    try:
        from antenv.axon_hooks import get_axon_ntff_profile_hook  # noqa: F401

        return
    except ImportError:
        pass
    import types

    import antenv

    try:
        from trn_agent_boot.trn_boot import _ntff_profile_via_ctypes
    except ImportError:
        return
    mod = types.ModuleType("antenv.axon_hooks")
    _hook = [None]
    mod.set_axon_ntff_profile_hook = lambda h: _hook.__setitem__(0, h)
    mod.get_axon_ntff_profile_hook = lambda: _hook[0]
    sys.modules["antenv.axon_hooks"] = mod
    antenv.axon_hooks = mod
    so = "/opt/axon/libaxon_pjrt.so"
    if os.path.exists(so):
        mod.set_axon_ntff_profile_hook(_ntff_profile_via_ctypes(so))


def _max_index_nd(nc, out, in_max, in_values):
    """max_index with a multi-dim in_values AP (bass asserts 2D, the DVE
    ISA takes general APs; indices are in AP traversal order)."""
    eng = nc.vector
    return eng.add_instruction(
        mybir.InstMaxIndex(
            name=nc.get_next_instruction_name(),
            ins=[eng.lower_ap(in_max), eng.lower_ap(in_values)],
            outs=[eng.lower_ap(out)],
        )
    )


def build_nc(use_act_copy: bool = False) -> bass.Bass:
    nc = bass.Bass()

    xg = nc.declare_dram_parameter("xg", [GROUPS, P, DCH * TPG * P], F16, isOutput=False)
    cons = nc.declare_dram_parameter("cons", [P, DCH * K + K + P], F16, isOutput=False)
    biasf = nc.declare_dram_parameter("biasf", [P, K], F32, isOutput=False)
    out = nc.declare_dram_parameter("out", [P, TILES], I32, isOutput=True)

    FLUSH = [16, 32, 48, 64, 80, 96, 112, 120, 126, 128]

    with TileContext(nc) as tc:
        with (
            tc.tile_pool(name="const", bufs=1) as constp,
            tc.tile_pool(name="xp0", bufs=DCH) as xp0,
            tc.tile_pool(name="xp", bufs=3) as xp,
            tc.tile_pool(name="mx", bufs=8) as mxp,
            tc.tile_pool(name="mx8", bufs=4) as mx8p,
            tc.tile_pool(name="psA", bufs=2, space="PSUM") as psp,
            tc.tile_pool(name="psB", bufs=3, space="PSUM") as ps2p,
            tc.tile_pool(name="outp", bufs=1) as outp,
        ):
            cons_t = constp.tile([P, DCH * K + K + P], F16)
            nc.scalar.dma_start(out=cons_t[:, DCH * K :], in_=cons[:, DCH * K :])
            nc.scalar.dma_start(out=cons_t[:, :K], in_=cons[:, :K])
            nc.scalar.dma_start(out=cons_t[:, K : DCH * K], in_=cons[:, K : DCH * K])
            ctiles = [cons_t[:, j * K : (j + 1) * K] for j in range(DCH)]
            btile = cons_t[:, DCH * K : DCH * K + K]
            otile = cons_t[:, DCH * K + K :]
            bft = constp.tile([P, K], F32)
            nc.scalar.dma_start(out=bft[:], in_=biasf[:])
            # doubled bias for the one-ACTIVATE-per-pair copy
            bft2 = constp.tile([P, 2, K], F32)
            for h in range(2):
                nc.scalar.copy(out=bft2[:, h, :], in_=bft[:])

            xch0 = []
            for j in range(DCH):
                cbuf = xp0.tile([P, TPG, P], F16, name="xchunk")
                nc.sync.dma_start(
                    out=cbuf[:],
                    in_=xg[0, :, j * TPG * P : (j + 1) * TPG * P].rearrange(
                        "p (t q) -> p t q", t=TPG
                    ),
                )
                xch0.append(cbuf)

            idxbuf = outp.tile([P, TILES, 8], U32)
            tokbuf = outp.tile([P, TILES], I32)

            def emit_flush(lo, hi):
                # Pool extracts tokens for tiles [lo, hi), scalar ring
                # DMAs them out. Singles (t<8) sit at slot t lane 0;
                # pair tiles sit at the pair's even slot, lanes 0/4
                # (odd-tile index is offset by 300, un-offset wrap-safe)
                if lo < 8:
                    se = min(hi, 8)
                    nc.gpsimd.tensor_copy(
                        out=tokbuf[:, lo:se], in_=idxbuf[:, lo:se, 0]
                    )
                p0 = max(lo, 8)
                if hi > 8:
                    nc.gpsimd.tensor_copy(
                        out=tokbuf[:, p0:hi:2], in_=idxbuf[:, p0:hi:2, 0]
                    )
                    nc.gpsimd.tensor_scalar(
                        tokbuf[:, p0 + 1 : hi : 2],
                        idxbuf[:, p0:hi:2, 4],
                        300,
                        300,
                        mybir.AluOpType.max,
                        mybir.AluOpType.subtract,
                    )
                nc.scalar.dma_start(out=out[:, lo:hi], in_=tokbuf[:, lo:hi])

            pending = None
            for g in range(GROUPS):
                if g == 0:
                    chunk = lambda j, tl: xch0[j][:, tl, :]
                else:
                    xbuf = xp.tile([P, DCH, TPG, P], F16, name="xgrp")
                    nc.sync.dma_start(
                        out=xbuf[:],
                        in_=xg[g].rearrange("p (j t q) -> p j t q", j=DCH, t=TPG),
                    )
                    chunk = lambda j, tl, xbuf=xbuf: xbuf[:, j, tl, :]
                if g < 2:
                    # tiles 0-7: baseline per-tile path (bias matmul
                    # start=True sets has_written; rate-matched to the
                    # chunk-major arrival stream while DMA is saturated)
                    for tl in range(TPG):
                        t = g * TPG + tl
                        psum = psp.tile([P, K], F32, name="ps1")
                        nc.tensor.matmul(
                            psum[:], lhsT=otile[:], rhs=btile[:],
                            start=True, stop=False,
                        )
                        for j in range(DCH):
                            nc.tensor.matmul(
                                psum[:],
                                lhsT=chunk(j, tl),
                                rhs=ctiles[j][:],
                                start=False,
                                stop=(j == DCH - 1),
                            )
                        mx = mxp.tile([P, 8], F32)
                        nc.vector.max(out=mx[:], in_=psum[:])
                        nc.vector.max_index(
                            out=idxbuf[:, t, :], in_max=mx[:], in_values=psum[:]
                        )
                    continue
                # tiles 8+: PAIRS sharing one 2-bank PSUM tile - one
                # 600-wide FIND per two tiles keeps DVE under the PE's
                # 2.03us/pair budget (per-tile FIND made DVE co-critical)
                for pl in range(TPG // 2):
                    pr = (g - 2) * (TPG // 2) + pl
                    base_t = 8 + 2 * pr
                    if pending is not None:
                        emit_flush(*pending)
                        pending = None
                    ps = ps2p.tile([P, 2, 512], F32, name="ps2")
                    if pr < 3:
                        # fresh banks: bias matmuls set has_written
                        for h in range(2):
                            nc.tensor.matmul(
                                ps[:, h, :K], lhsT=otile[:], rhs=btile[:],
                                start=True, stop=False,
                            )
                    else:
                        # bits persist from the bank's previous pair;
                        # one ACTIVATE rewrites both halves to the bias
                        nc.scalar.copy(out=ps[:, :, :K], in_=bft2[:])
                    mxt = mxp.tile([P, 2, 8], F32, name="mxt")
                    for h in range(2):
                        for j in range(DCH):
                            nc.tensor.matmul(
                                ps[:, h, :K],
                                lhsT=chunk(j, 2 * pl + h),
                                rhs=ctiles[j][:],
                                start=False,
                                stop=(j == DCH - 1),
                                skip_group_check=(pr >= 3),
                            )
                        nc.vector.max(out=mxt[:, h, :], in_=ps[:, h, :K])
                    # Pool packs the FIND keys [A0..A3, B0..B3]; lanes 0
                    # (maxA) and 4 (maxB) are consumed, the rest benign
                    mx8 = mx8p.tile([P, 8], F32)
                    nc.gpsimd.tensor_copy(out=mx8[:], in_=mxt[:, :, 0:4])
                    _max_index_nd(
                        nc, out=idxbuf[:, base_t, :], in_max=mx8[:],
                        in_values=ps[:, :, :K],
                    )
                    # defer the flush one pair: emitting it here would
                    # park the token-DMA issue in the scalar queue ahead
                    # of the next pair's bias ACTIVATE and stall the PE
                    e = base_t + 2
                    if e in FLUSH:
                        lo = FLUSH[FLUSH.index(e) - 1] if e != FLUSH[0] else 0
                        if e == TILES:
                            emit_flush(lo, e)
                        else:
                            pending = (lo, e)

    _hoist_excess_waits(nc)
    return nc


def _hoist_excess_waits(nc: bass.Bass, max_waits: int = 1):
    n = 0
    for f in nc.m.functions:
        for blk in f.blocks:
            insts = blk.instructions
            i = 0
            while i < len(insts):
                inst = insts[i]
                si = inst.sync_info
                if si and si.on_wait and len(si.on_wait) > max_waits:
                    waits = list(si.on_wait)
                    si.on_wait = waits[-max_waits:]
                    inst.sync_info = si
                    pre = []
                    for j in range(0, len(waits) - max_waits, max_waits):
                        nd = mybir.InstNoOp(name=f"I-wsplit{n}", ins=[], outs=[])
                        n += 1
                        nd.engine = inst.engine
                        nsi = type(si)(
                            on_wait=waits[j : j + max_waits], on_update=[]
                        )
                        nd.sync_info = nsi
                        try:
                            nc.register_instruction(nd, overwrite=True)
                        except Exception:
                            pass
                        pre.append(nd)
                    for k, nd in enumerate(pre):
                        insts.insert(i + k, nd)
                    i += len(pre)
                i += 1


def make_in_maps(x, C, Cnorm):
    x16 = x.astype(np.float16)
    C16 = C.astype(np.float16).reshape(DCH, P, K)

    bz = (-0.5 * Cnorm.reshape(K)).astype(np.float32)
    bh = bz.astype(np.float16)
    bl = (bz - bh.astype(np.float32)).astype(np.float16)

    cons = np.zeros((P, DCH * K + K + P), np.float16)
    cons[:, : DCH * K] = C16.transpose(1, 0, 2).reshape(P, DCH * K)
    cons[0, DCH * K : DCH * K + K] = bh
    cons[1, DCH * K : DCH * K + K] = bl
    cons[0:2, DCH * K + K :] = 1.0

    biasf = np.broadcast_to(
        bh.astype(np.float32) + bl.astype(np.float32), (P, K)
    ).copy()

    in_maps = []
    for c in range(NCORES):
        xs = x16[c * ROWS : (c + 1) * ROWS]
        xr = xs.reshape(P, GROUPS, TPG, DCH, P)
        xgc = np.ascontiguousarray(xr.transpose(1, 4, 3, 2, 0))
        in_maps.append(
            {
                "xg": xgc.reshape(GROUPS, P, DCH * TPG * P),
                "cons": cons,
                "biasf": biasf,
            }
        )
    return in_maps


_NC_CACHE = {}


def kernel(x, C, Cnorm, b, t):
    global LAST_RESULT
    x = np.asarray(x)
    C = np.asarray(C)
    Cnorm = np.asarray(Cnorm)

    key = 0
    if key not in _NC_CACHE:
        _NC_CACHE[key] = build_nc()
    nc = _NC_CACHE[key]

    in_maps = make_in_maps(x, C, Cnorm)
    trace = bool(int(os.environ.get("KM_TRACE", "0")))
    if trace:
        _ensure_ntff_hook()
    res = run_bass_kernel_spmd(
        nc, in_maps, core_ids=list(range(NCORES)), trace=trace
    )
    LAST_RESULT = res

    shards = [res.results[c]["out"].reshape(-1) for c in range(NCORES)]
    tokens = np.concatenate(shards).astype(np.int32)
    return tokens.reshape(int(b), int(t))


# revision 27
# speedup vs baseline: 1.0826x; 1.0826x over previous
"""VQ codebook assignment (ApplyKmeans) on 8 Trainium2 NeuronCores.

tokens[n] = argmin_k ||x_n - c_k||^2 = argmax_k (x_n.c_k - Cnorm_k/2)
(||x_n||^2 is constant per row). Data-parallel: x sharded along N
across 8 cores, C/Cnorm replicated.

Per core (16384 rows, 128 row-tiles of 128 rows):
  - host pre-tiles x^T so each [128d, 128n] stationary tile is
    contiguous (fp16: halves HBM traffic; PSUM accumulates fp32;
    52/131072 argmin flips vs the fp32 reference, rel err 0.0144)
  - per tile: 8 accumulating matmuls (x^T chunk stationary, C chunk
    moving) on top of a bias pre-load, so PSUM [128, 300] holds
    val = x.C - Cnorm/2
  - bias pre-load: tiles 0-7 matmul the bias in (ones x [-Cnorm/2
    hi/lo] rows, start=True) which sets every PSUM has_written bit;
    afterwards the otherwise-idle ScalarE rewrites the bank to the
    bias values and the start=False matmuls accumulate onto it -
    saving the PE a 9th matmul per tile
  - VectorE max8 + max_index -> first-occurrence argmax per row
  - tokens assembled in SBUF, streamed out in 16-tile blocks through
    t=112, then 8/6/2 so the final CAST+DMA on the kernel tail is tiny

Row interleaving: row-tile t holds rows {p*128 + t}, so the token
buffer [p, t] DMAs out contiguously in original row order.

Walrus only lowers one sync wait per instruction; _hoist_excess_waits
moves Tile's extra waits onto same-engine no-ops at the same program
point. x loads share the sync HWDGE ring (same-ring transfers complete
in order, so prefetch can't starve urgent loads); constants and token
stores ride the scalar ring.

Optimization notes from profiling (kept for future work): steady-state
PE is ~130ns/matmul vs a 127.2 floor; DVE (max8 478 + find_index8 469
+ match 91 per tile) is co-critical with the PE. A pair-FIND variant
(two tiles per 2-bank PSUM tile, one 600-wide FIND) reaches 127ns/MM
steady but loses ~1us net: startup is DMA-bandwidth-bound (~330GB/s
shared across rings, ~2.2us first-transfer latency after the ~7.5us
framework preamble), and this layout''s chunk-major group-0 +
bias-matmul start is better rate-matched to the arrival stream.
Engine clock varies run to run (2.4 vs 2.0 GHz throttle episodes:
check MATMUL median duration ~290ns vs ~348ns in the profile before
comparing timings).
"""
import os
import sys

import numpy as np

if "/opt/trn_rl_repo" not in sys.path:
    sys.path.insert(0, "/opt/trn_rl_repo")

import concourse.bass as bass
import concourse.mybir as mybir
import concourse.tile_sem_assignment as _tsa
from concourse.bass_utils import run_bass_kernel_spmd
from concourse.tile import TileContext

_tsa.NUM_HWDGE_SEMS = int(os.environ.get("KM_HW_LANES", "8"))

_orig_assign_tick = _tsa.TileClockTick._assign_tick


def _assign_tick_lanepools(self, inst):
    try:
        if isinstance(inst, _tsa.DMAInst) and inst.engine != mybir.EngineType.Pool:
            if not hasattr(self, "_lane_ctr"):
                self._lane_ctr = {}
            eng = inst.engine
            n = _tsa.NUM_HWDGE_SEMS
            half = max(1, n // 2)
            pool = (
                list(range(0, half))
                if eng == mybir.EngineType.Activation
                else list(range(half, n))
            )
            c = self._lane_ctr.get(eng, 0)
            self.next_hw_dma_idx = pool[c % len(pool)]
            self._lane_ctr[eng] = c + 1
    except Exception:
        pass
    return _orig_assign_tick(self, inst)


_tsa.TileClockTick._assign_tick = _assign_tick_lanepools

P = 128
D = 1024
K = 300
NCORES = 8
ROWS = 16384
TILES = ROWS // P
GROUPS = 32
TPG = TILES // GROUPS
DCH = D // P

F16 = mybir.dt.float16
F32 = mybir.dt.float32
I32 = mybir.dt.int32
U32 = mybir.dt.uint32

LAST_RESULT = None


def _ensure_ntff_hook():
    try:
        from antenv.axon_hooks import get_axon_ntff_profile_hook  # noqa: F401

        return
    except ImportError:
        pass
    import types

    import antenv

    try:
        from trn_agent_boot.trn_boot import _ntff_profile_via_ctypes
    except ImportError:
        return
    mod = types.ModuleType("antenv.axon_hooks")
    _hook = [None]
    mod.set_axon_ntff_profile_hook = lambda h: _hook.__setitem__(0, h)
    mod.get_axon_ntff_profile_hook = lambda: _hook[0]
    sys.modules["antenv.axon_hooks"] = mod
    antenv.axon_hooks = mod
    so = "/opt/axon/libaxon_pjrt.so"
    if os.path.exists(so):
        mod.set_axon_ntff_profile_hook(_ntff_profile_via_ctypes(so))


def _max_index_nd(nc, out, in_max, in_values):
    """max_index with a multi-dim in_values AP (bass asserts 2D, the DVE
    ISA takes general APs; indices are in AP traversal order)."""
    eng = nc.vector
    return eng.add_instruction(
        mybir.InstMaxIndex(
            name=nc.get_next_instruction_name(),
            ins=[eng.lower_ap(in_max), eng.lower_ap(in_values)],
            outs=[eng.lower_ap(out)],
        )
    )


def build_nc(use_act_copy: bool = False) -> bass.Bass:
    nc = bass.Bass()

    xg = nc.declare_dram_parameter("xg", [GROUPS, P, DCH * TPG * P], F16, isOutput=False)
    cons = nc.declare_dram_parameter("cons", [P, DCH * K + K + P], F16, isOutput=False)
    biasf = nc.declare_dram_parameter("biasf", [P, K], F32, isOutput=False)
    out = nc.declare_dram_parameter("out", [P, TILES], I32, isOutput=True)

    FLUSH = [16, 32, 48, 64, 80, 96, 112, 120, 126, 128]

    with TileContext(nc) as tc:
        with (
            tc.tile_pool(name="const", bufs=1) as constp,
            tc.tile_pool(name="xp0", bufs=DCH) as xp0,
            tc.tile_pool(name="xp", bufs=3) as xp,
            tc.tile_pool(name="mx", bufs=8) as mxp,
            tc.tile_pool(name="mx8", bufs=4) as mx8p,
            tc.tile_pool(name="psum", bufs=4, space="PSUM") as ps2p,
            tc.tile_pool(name="outp", bufs=1) as outp,
        ):
            cons_t = constp.tile([P, DCH * K + K + P], F16)
            nc.scalar.dma_start(out=cons_t[:, DCH * K :], in_=cons[:, DCH * K :])
            nc.scalar.dma_start(out=cons_t[:, :K], in_=cons[:, :K])
            nc.scalar.dma_start(out=cons_t[:, K : DCH * K], in_=cons[:, K : DCH * K])
            ctiles = [cons_t[:, j * K : (j + 1) * K] for j in range(DCH)]
            btile = cons_t[:, DCH * K : DCH * K + K]
            otile = cons_t[:, DCH * K + K :]
            bft = constp.tile([P, K], F32)
            nc.scalar.dma_start(out=bft[:], in_=biasf[:])
            # doubled bias for the one-ACTIVATE-per-pair copy
            bft2 = constp.tile([P, 2, K], F32)
            for h in range(2):
                nc.scalar.copy(out=bft2[:, h, :], in_=bft[:])

            xch0 = []
            for j in range(DCH):
                cbuf = xp0.tile([P, TPG, P], F16, name="xchunk")
                nc.sync.dma_start(
                    out=cbuf[:],
                    in_=xg[0, :, j * TPG * P : (j + 1) * TPG * P].rearrange(
                        "p (t q) -> p t q", t=TPG
                    ),
                )
                xch0.append(cbuf)

            idxbuf = outp.tile([P, TILES, 8], U32)
            tokbuf = outp.tile([P, TILES], I32)

            def emit_flush(lo, hi):
                # Pool extracts tokens for tiles [lo, hi), scalar ring
                # DMAs them out. Singles (t<8) sit at slot t lane 0;
                # pair tiles sit at the pair's even slot, lanes 0/4
                # (odd-tile index is offset by 300, un-offset wrap-safe)
                if lo < 8:
                    se = min(hi, 8)
                    nc.gpsimd.tensor_copy(
                        out=tokbuf[:, lo:se], in_=idxbuf[:, lo:se, 0]
                    )
                p0 = max(lo, 8)
                if hi > 8:
                    nc.gpsimd.tensor_copy(
                        out=tokbuf[:, p0:hi:2], in_=idxbuf[:, p0:hi:2, 0]
                    )
                    nc.gpsimd.tensor_scalar(
                        tokbuf[:, p0 + 1 : hi : 2],
                        idxbuf[:, p0:hi:2, 4],
                        300,
                        300,
                        mybir.AluOpType.max,
                        mybir.AluOpType.subtract,
                    )
                nc.scalar.dma_start(out=out[:, lo:hi], in_=tokbuf[:, lo:hi])

            # the 8 startup singles live in the halves of the 4 pair
            # tiles: their start=True bias matmuls set every half-bank's
            # has_written bits, so all pairs use the ACT-copy bias path
            # and the pair pipeline gets full 4-tile elasticity
            wtiles = [ps2p.tile([P, 2, 512], F32, name="ps2") for _ in range(4)]
            pending = None
            for g in range(GROUPS):
                if g == 0:
                    chunk = lambda j, tl: xch0[j][:, tl, :]
                else:
                    xbuf = xp.tile([P, DCH, TPG, P], F16, name="xgrp")
                    nc.sync.dma_start(
                        out=xbuf[:],
                        in_=xg[g].rearrange("p (j t q) -> p j t q", j=DCH, t=TPG),
                    )
                    chunk = lambda j, tl, xbuf=xbuf: xbuf[:, j, tl, :]
                if g < 2:
                    # tiles 0-7: baseline per-tile path (bias matmul
                    # start=True sets has_written; rate-matched to the
                    # chunk-major arrival stream while DMA is saturated)
                    for tl in range(TPG):
                        t = g * TPG + tl
                        psum = wtiles[t // 2][:, t % 2, :K]
                        nc.tensor.matmul(
                            psum[:], lhsT=otile[:], rhs=btile[:],
                            start=True, stop=False,
                        )
                        for j in range(DCH):
                            nc.tensor.matmul(
                                psum[:],
                                lhsT=chunk(j, tl),
                                rhs=ctiles[j][:],
                                start=False,
                                stop=(j == DCH - 1),
                            )
                        mx = mxp.tile([P, 8], F32)
                        nc.vector.max(out=mx[:], in_=psum[:])
                        nc.vector.max_index(
                            out=idxbuf[:, t, :], in_max=mx[:], in_values=psum[:]
                        )
                    continue
                # tiles 8+: PAIRS sharing one 2-bank PSUM tile - one
                # 600-wide FIND per two tiles keeps DVE under the PE's
                # 2.03us/pair budget (per-tile FIND made DVE co-critical)
                for pl in range(TPG // 2):
                    pr = (g - 2) * (TPG // 2) + pl
                    base_t = 8 + 2 * pr
                    if pending is not None:
                        emit_flush(*pending)
                        pending = None
                    ps = ps2p.tile([P, 2, 512], F32, name="ps2")
                    # bits persist from the bank's previous occupant
                    # (startup single or prior pair); one ACTIVATE
                    # rewrites both halves to the bias
                    nc.scalar.copy(out=ps[:, :, :K], in_=bft2[:])
                    mxt = mxp.tile([P, 2, 8], F32, name="mxt")
                    for h in range(2):
                        for j in range(DCH):
                            nc.tensor.matmul(
                                ps[:, h, :K],
                                lhsT=chunk(j, 2 * pl + h),
                                rhs=ctiles[j][:],
                                start=False,
                                stop=(j == DCH - 1),
                                skip_group_check=True,
                            )
                        nc.vector.max(out=mxt[:, h, :], in_=ps[:, h, :K])
                    # Pool packs the FIND keys [A0..A3, B0..B3]; lanes 0
                    # (maxA) and 4 (maxB) are consumed, the rest benign
                    mx8 = mx8p.tile([P, 8], F32)
                    nc.gpsimd.tensor_copy(out=mx8[:], in_=mxt[:, :, 0:4])
                    _max_index_nd(
                        nc, out=idxbuf[:, base_t, :], in_max=mx8[:],
                        in_values=ps[:, :, :K],
                    )
                    # defer the flush one pair: emitting it here would
                    # park the token-DMA issue in the scalar queue ahead
                    # of the next pair's bias ACTIVATE and stall the PE
                    e = base_t + 2
                    if e in FLUSH:
                        lo = FLUSH[FLUSH.index(e) - 1] if e != FLUSH[0] else 0
                        if e == TILES:
                            emit_flush(lo, e)
                        else:
                            pending = (lo, e)

    _hoist_excess_waits(nc)
    return nc


def _hoist_excess_waits(nc: bass.Bass, max_waits: int = 1):
    n = 0
    for f in nc.m.functions:
        for blk in f.blocks:
            insts = blk.instructions
            i = 0
            while i < len(insts):
                inst = insts[i]
                si = inst.sync_info
                if si and si.on_wait and len(si.on_wait) > max_waits:
                    waits = list(si.on_wait)
                    si.on_wait = waits[-max_waits:]
                    inst.sync_info = si
                    pre = []
                    for j in range(0, len(waits) - max_waits, max_waits):
                        nd = mybir.InstNoOp(name=f"I-wsplit{n}", ins=[], outs=[])
                        n += 1
                        nd.engine = inst.engine
                        nsi = type(si)(
                            on_wait=waits[j : j + max_waits], on_update=[]
                        )
                        nd.sync_info = nsi
                        try:
                            nc.register_instruction(nd, overwrite=True)
                        except Exception:
                            pass
                        pre.append(nd)
                    for k, nd in enumerate(pre):
                        insts.insert(i + k, nd)
                    i += len(pre)
                i += 1


def make_in_maps(x, C, Cnorm):
    x16 = x.astype(np.float16)
    C16 = C.astype(np.float16).reshape(DCH, P, K)

    bz = (-0.5 * Cnorm.reshape(K)).astype(np.float32)
    bh = bz.astype(np.float16)
    bl = (bz - bh.astype(np.float32)).astype(np.float16)

    cons = np.zeros((P, DCH * K + K + P), np.float16)
    cons[:, : DCH * K] = C16.transpose(1, 0, 2).reshape(P, DCH * K)
    cons[0, DCH * K : DCH * K + K] = bh
    cons[1, DCH * K : DCH * K + K] = bl
    cons[0:2, DCH * K + K :] = 1.0

    biasf = np.broadcast_to(
        bh.astype(np.float32) + bl.astype(np.float32), (P, K)
    ).copy()

    in_maps = []
    for c in range(NCORES):
        xs = x16[c * ROWS : (c + 1) * ROWS]
        xr = xs.reshape(P, GROUPS, TPG, DCH, P)
        xgc = np.ascontiguousarray(xr.transpose(1, 4, 3, 2, 0))
        in_maps.append(
            {
                "xg": xgc.reshape(GROUPS, P, DCH * TPG * P),
                "cons": cons,
                "biasf": biasf,
            }
        )
    return in_maps


_NC_CACHE = {}


def kernel(x, C, Cnorm, b, t):
    global LAST_RESULT
    x = np.asarray(x)
    C = np.asarray(C)
    Cnorm = np.asarray(Cnorm)

    key = 0
    if key not in _NC_CACHE:
        _NC_CACHE[key] = build_nc()
    nc = _NC_CACHE[key]

    in_maps = make_in_maps(x, C, Cnorm)
    trace = bool(int(os.environ.get("KM_TRACE", "0")))
    if trace:
        _ensure_ntff_hook()
    res = run_bass_kernel_spmd(
        nc, in_maps, core_ids=list(range(NCORES)), trace=trace
    )
    LAST_RESULT = res

    shards = [res.results[c]["out"].reshape(-1) for c in range(NCORES)]
    tokens = np.concatenate(shards).astype(np.int32)
    return tokens.reshape(int(b), int(t))


# revision 28
# speedup vs baseline: 1.1123x; 1.0274x over previous
"""VQ codebook assignment (ApplyKmeans) on 8 Trainium2 NeuronCores.

tokens[n] = argmin_k ||x_n - c_k||^2 = argmax_k (x_n.c_k - Cnorm_k/2)
(||x_n||^2 is constant per row). Data-parallel: x sharded along N
across 8 cores, C/Cnorm replicated.

Per core (16384 rows, 128 row-tiles of 128 rows):
  - host pre-tiles x^T so each [128d, 128n] stationary tile is
    contiguous (fp16: halves HBM traffic; PSUM accumulates fp32;
    52/131072 argmin flips vs the fp32 reference, rel err 0.0144)
  - per tile: 8 accumulating matmuls (x^T chunk stationary, C chunk
    moving) on top of a bias pre-load, so PSUM [128, 300] holds
    val = x.C - Cnorm/2
  - bias pre-load: tiles 0-7 matmul the bias in (ones x [-Cnorm/2
    hi/lo] rows, start=True) which sets every PSUM has_written bit;
    afterwards the otherwise-idle ScalarE rewrites the bank to the
    bias values and the start=False matmuls accumulate onto it -
    saving the PE a 9th matmul per tile
  - VectorE max8 + max_index -> first-occurrence argmax per row
  - tokens assembled in SBUF, streamed out in 16-tile blocks through
    t=112, then 8/6/2 so the final CAST+DMA on the kernel tail is tiny

Row interleaving: row-tile t holds rows {p*128 + t}, so the token
buffer [p, t] DMAs out contiguously in original row order.

Walrus only lowers one sync wait per instruction; _hoist_excess_waits
moves Tile's extra waits onto same-engine no-ops at the same program
point. x loads share the sync HWDGE ring (same-ring transfers complete
in order, so prefetch can't starve urgent loads); constants and token
stores ride the scalar ring.

Optimization notes from profiling (kept for future work): steady-state
PE is ~130ns/matmul vs a 127.2 floor; DVE (max8 478 + find_index8 469
+ match 91 per tile) is co-critical with the PE. A pair-FIND variant
(two tiles per 2-bank PSUM tile, one 600-wide FIND) reaches 127ns/MM
steady but loses ~1us net: startup is DMA-bandwidth-bound (~330GB/s
shared across rings, ~2.2us first-transfer latency after the ~7.5us
framework preamble), and this layout''s chunk-major group-0 +
bias-matmul start is better rate-matched to the arrival stream.
Engine clock varies run to run (2.4 vs 2.0 GHz throttle episodes:
check MATMUL median duration ~290ns vs ~348ns in the profile before
comparing timings).
"""
import os
import sys

import numpy as np

if "/opt/trn_rl_repo" not in sys.path:
    sys.path.insert(0, "/opt/trn_rl_repo")

import concourse.bass as bass
import concourse.mybir as mybir
import concourse.tile_sem_assignment as _tsa
from concourse.bass_utils import run_bass_kernel_spmd
from concourse.tile import TileContext

_tsa.NUM_HWDGE_SEMS = int(os.environ.get("KM_HW_LANES", "8"))

_orig_assign_tick = _tsa.TileClockTick._assign_tick


def _assign_tick_lanepools(self, inst):
    try:
        if isinstance(inst, _tsa.DMAInst) and inst.engine != mybir.EngineType.Pool:
            if not hasattr(self, "_lane_ctr"):
                self._lane_ctr = {}
            eng = inst.engine
            n = _tsa.NUM_HWDGE_SEMS
            half = max(1, n // 2)
            pool = (
                list(range(0, half))
                if eng == mybir.EngineType.Activation
                else list(range(half, n))
            )
            c = self._lane_ctr.get(eng, 0)
            self.next_hw_dma_idx = pool[c % len(pool)]
            self._lane_ctr[eng] = c + 1
    except Exception:
        pass
    return _orig_assign_tick(self, inst)


_tsa.TileClockTick._assign_tick = _assign_tick_lanepools

P = 128
D = 1024
K = 300
NCORES = 8
ROWS = 16384
TILES = ROWS // P
GROUPS = 32
TPG = TILES // GROUPS
DCH = D // P

F16 = mybir.dt.float16
F32 = mybir.dt.float32
I32 = mybir.dt.int32
U32 = mybir.dt.uint32

LAST_RESULT = None


def _ensure_ntff_hook():
    try:
        from antenv.axon_hooks import get_axon_ntff_profile_hook  # noqa: F401

        return
    except ImportError:
        pass
    import types

    import antenv

    try:
        from trn_agent_boot.trn_boot import _ntff_profile_via_ctypes
    except ImportError:
        return
    mod = types.ModuleType("antenv.axon_hooks")
    _hook = [None]
    mod.set_axon_ntff_profile_hook = lambda h: _hook.__setitem__(0, h)
    mod.get_axon_ntff_profile_hook = lambda: _hook[0]
    sys.modules["antenv.axon_hooks"] = mod
    antenv.axon_hooks = mod
    so = "/opt/axon/libaxon_pjrt.so"
    if os.path.exists(so):
        mod.set_axon_ntff_profile_hook(_ntff_profile_via_ctypes(so))


def build_nc(use_act_copy: bool = False) -> bass.Bass:
    nc = bass.Bass()

    xg = nc.declare_dram_parameter("xg", [GROUPS, P, DCH * TPG * P], F16, isOutput=False)
    cons = nc.declare_dram_parameter("cons", [P, DCH * K + K + P], F16, isOutput=False)
    biasf = nc.declare_dram_parameter("biasf", [P, K], F32, isOutput=False)
    out = nc.declare_dram_parameter("out", [P, TILES], I32, isOutput=True)

    FLUSH = [16, 32, 48, 64, 80, 96, 112, 120, 126, 128]

    with TileContext(nc) as tc:
        with (
            tc.tile_pool(name="const", bufs=1) as constp,
            tc.tile_pool(name="xp0", bufs=DCH) as xp0,
            tc.tile_pool(name="xp", bufs=3) as xp,
            tc.tile_pool(name="mx", bufs=8) as mxp,
            tc.tile_pool(name="val", bufs=4) as valp,
            tc.tile_pool(name="psum", bufs=8, space="PSUM") as psp,
            tc.tile_pool(name="outp", bufs=1) as outp,
        ):
            cons_t = constp.tile([P, DCH * K + K + P], F16)
            nc.scalar.dma_start(out=cons_t[:, DCH * K :], in_=cons[:, DCH * K :])
            nc.scalar.dma_start(out=cons_t[:, :K], in_=cons[:, :K])
            nc.scalar.dma_start(out=cons_t[:, K : DCH * K], in_=cons[:, K : DCH * K])
            ctiles = [cons_t[:, j * K : (j + 1) * K] for j in range(DCH)]
            btile = cons_t[:, DCH * K : DCH * K + K]
            otile = cons_t[:, DCH * K + K :]
            bft = constp.tile([P, K], F32)
            nc.scalar.dma_start(out=bft[:], in_=biasf[:])

            xch0 = []
            for j in range(DCH):
                cbuf = xp0.tile([P, TPG, P], F16, name="xchunk")
                nc.sync.dma_start(
                    out=cbuf[:],
                    in_=xg[0, :, j * TPG * P : (j + 1) * TPG * P].rearrange(
                        "p (t q) -> p t q", t=TPG
                    ),
                )
                xch0.append(cbuf)

            idxbuf = outp.tile([P, TILES, 8], U32)
            tokbuf = outp.tile([P, TILES], I32)

            for g in range(GROUPS):
                if g == 0:
                    chunk = lambda j, tl: xch0[j][:, tl, :]
                else:
                    xbuf = xp.tile([P, DCH, TPG, P], F16, name="xgrp")
                    nc.sync.dma_start(
                        out=xbuf[:],
                        in_=xg[g].rearrange("p (j t q) -> p j t q", j=DCH, t=TPG),
                    )
                    chunk = lambda j, tl, xbuf=xbuf: xbuf[:, j, tl, :]
                for tl in range(TPG):
                    t = g * TPG + tl
                    psum = psp.tile([P, K], F32)
                    if t < 8:
                        nc.tensor.matmul(
                            psum[:], lhsT=otile[:], rhs=btile[:],
                            start=True, stop=False,
                        )
                    else:
                        nc.scalar.copy(out=psum[:], in_=bft[:])
                    for j in range(DCH):
                        nc.tensor.matmul(
                            psum[:],
                            lhsT=chunk(j, tl),
                            rhs=ctiles[j][:],
                            start=False,
                            stop=(j == DCH - 1),
                            skip_group_check=(t >= 8),
                        )
                    src = psum
                    mx = mxp.tile([P, 8], F32)
                    nc.vector.max(out=mx[:], in_=src[:])
                    nc.vector.max_index(
                        out=idxbuf[:, t, :], in_max=mx[:], in_values=src[:]
                    )
                    if (t + 1) in FLUSH:
                        s = FLUSH[FLUSH.index(t + 1) - 1] if (t + 1) != FLUSH[0] else 0
                        nc.vector.tensor_copy(
                            out=tokbuf[:, s : t + 1], in_=idxbuf[:, s : t + 1, 0]
                        )
                        nc.scalar.dma_start(
                            out=out[:, s : t + 1], in_=tokbuf[:, s : t + 1]
                        )

    _hoist_excess_waits(nc)
    return nc


def _hoist_excess_waits(nc: bass.Bass, max_waits: int = 1):
    n = 0
    for f in nc.m.functions:
        for blk in f.blocks:
            insts = blk.instructions
            i = 0
            while i < len(insts):
                inst = insts[i]
                si = inst.sync_info
                if si and si.on_wait and len(si.on_wait) > max_waits:
                    waits = list(si.on_wait)
                    si.on_wait = waits[-max_waits:]
                    inst.sync_info = si
                    pre = []
                    for j in range(0, len(waits) - max_waits, max_waits):
                        nd = mybir.InstNoOp(name=f"I-wsplit{n}", ins=[], outs=[])
                        n += 1
                        nd.engine = inst.engine
                        nsi = type(si)(
                            on_wait=waits[j : j + max_waits], on_update=[]
                        )
                        nd.sync_info = nsi
                        try:
                            nc.register_instruction(nd, overwrite=True)
                        except Exception:
                            pass
                        pre.append(nd)
                    for k, nd in enumerate(pre):
                        insts.insert(i + k, nd)
                    i += len(pre)
                i += 1


def make_in_maps(x, C, Cnorm):
    x16 = x.astype(np.float16)
    C16 = C.astype(np.float16).reshape(DCH, P, K)

    bz = (-0.5 * Cnorm.reshape(K)).astype(np.float32)
    bh = bz.astype(np.float16)
    bl = (bz - bh.astype(np.float32)).astype(np.float16)

    cons = np.zeros((P, DCH * K + K + P), np.float16)
    cons[:, : DCH * K] = C16.transpose(1, 0, 2).reshape(P, DCH * K)
    cons[0, DCH * K : DCH * K + K] = bh
    cons[1, DCH * K : DCH * K + K] = bl
    cons[0:2, DCH * K + K :] = 1.0

    biasf = np.broadcast_to(
        bh.astype(np.float32) + bl.astype(np.float32), (P, K)
    ).copy()

    in_maps = []
    for c in range(NCORES):
        xs = x16[c * ROWS : (c + 1) * ROWS]
        xr = xs.reshape(P, GROUPS, TPG, DCH, P)
        xgc = np.ascontiguousarray(xr.transpose(1, 4, 3, 2, 0))
        in_maps.append(
            {
                "xg": xgc.reshape(GROUPS, P, DCH * TPG * P),
                "cons": cons,
                "biasf": biasf,
            }
        )
    return in_maps


_NC_CACHE = {}


def kernel(x, C, Cnorm, b, t):
    global LAST_RESULT
    x = np.asarray(x)
    C = np.asarray(C)
    Cnorm = np.asarray(Cnorm)

    key = 0
    if key not in _NC_CACHE:
        _NC_CACHE[key] = build_nc()
    nc = _NC_CACHE[key]

    in_maps = make_in_maps(x, C, Cnorm)
    trace = bool(int(os.environ.get("KM_TRACE", "0")))
    if trace:
        _ensure_ntff_hook()
    res = run_bass_kernel_spmd(
        nc, in_maps, core_ids=list(range(NCORES)), trace=trace
    )
    LAST_RESULT = res

    shards = [res.results[c]["out"].reshape(-1) for c in range(NCORES)]
    tokens = np.concatenate(shards).astype(np.int32)
    return tokens.reshape(int(b), int(t))
